# revision 29
# baseline (speedup 1.0000x reference)
"""Multi-head attention (B=4, S=2048, D=1024, H=16, HD=64) on 8 TRN2 cores.

Sharding: batch (4) x head-halves (2) -> 8 cores; core c handles batch c//2
and heads [8*(c%2), 8*(c%2)+8) (512 columns of every projection). No
cross-core communication.

Per-core kernel (Tile framework):
  - X (Q/K/V batch slices) ingested via SWDGE cast-DMA (fp32->bf16 on the
    fly, Pool ring), then transposed to d-major with xbar transpose DMAs
    split across the SP and DVE rings.
  - Projections on TensorE in bf16: qT/kT feature-major [head-pair cols,
    toks]; V token-major with a ones column interleaved per head
    ([v_h | 1] is the PV stationary operand, so softmax denominators fall
    out of the PV matmul for free). V is stored fp8e4.
  - scores^T = kT.T @ qT per 128-token k-chunk (bf16), two heads packed in
    the PE array via row tiling.
  - softmax: exp on ScalarE straight out of PSUM (scale=1/8 folded in, no
    max-subtraction -- scores are O(1) by construction), fp8e4 P^T to SBUF.
  - PV: fp8 DoubleRow matmuls contract 256 tokens (a k-chunk pair) per
    instruction at 0.5 cyc/row; out^T[hd+1, q] accumulates in PSUM; row 64
    is the denom.
  - finalize: PE-transpose to token-major, multiply by reciprocal denom, DMA.
  - Schedule: K/V blocks ingest+project first; Q blocks pipeline under the
    attention phase so the ACT exp stream (the bottleneck) starts early and
    stays saturated.
"""

import numpy as np

import concourse.bass as bass
import concourse.tile as tile
from concourse import mybir
from concourse.masks import make_identity

B, S, D_IN, D_MODEL, H = 4, 2048, 1024, 1024, 16
HD = D_MODEL // H  # 64
N_CORES = 8
COLS = 512  # per-core projection columns (8 heads)
NPAIR = 4  # head pairs per core
NKC = S // 128  # 16 k-chunks
NDC = D_IN // 128  # 8 d_in chunks
QB = 512  # q block
NQB = S // QB  # 4

F32 = mybir.dt.float32
BF16 = mybir.dt.bfloat16
FP16 = mybir.dt.float16
FP8 = mybir.dt.float8e4
EXP = mybir.ActivationFunctionType.Exp
DR = mybir.MatmulPerfMode.DoubleRow


def _fixup_multi_waits(nc):
    """Split >cap sync waits per instruction into preceding same-engine NoOps.

    This walrus build rejects more than 1 sync wait command per instruction
    (2 for EventSemaphore); Tile's drain/backedge paths can attach one wait
    per live semaphore to a single Drain.
    """
    for fn in nc.m.functions:
        for block in fn.blocks:
            insts = block.instructions
            i = 0
            while i < len(insts):
                inst = insts[i]
                si = inst.sync_info
                cap = 2 if isinstance(inst, mybir.InstEventSemaphore) else 1
                if si is not None and len(si.on_wait) > cap:
                    waits = list(si.on_wait)
                    keep, extra = waits[:cap], waits[cap:]
                    inst.sync_info = mybir.SyncInfo(
                        on_wait=keep, on_update=list(si.on_update)
                    )
                    nops = [
                        mybir.InstNoOp(
                            name=f"{inst.name}_xwait{j}",
                            engine=inst.engine,
                            bass_nofuse=True,
                            sync_info=mybir.SyncInfo(on_wait=[w], on_update=[]),
                        )
                        for j, w in enumerate(extra)
                    ]
                    insts[i:i] = nops
                    i += len(nops)
                i += 1


class _TC(tile.TileContext):
    def __exit__(self, *args):
        ret = super().__exit__(*args)
        _fixup_multi_waits(self.nc)
        return ret


def build_core_program(fixup_waits: bool = True, time_reps: int = 1, prologue_only: bool = False, ablate: str = '') -> bass.Bass:
    tc_cls = _TC if fixup_waits else tile.TileContext
    nc = bass.Bass()
    xq = nc.dram_tensor("xq", [S, D_IN], F32, kind="ExternalInput")
    xk = nc.dram_tensor("xk", [S, D_IN], F32, kind="ExternalInput")
    xv = nc.dram_tensor("xv", [S, D_IN], F32, kind="ExternalInput")
    wq = nc.dram_tensor("wq", [D_IN, COLS], F32, kind="ExternalInput")
    wk = nc.dram_tensor("wk", [D_IN, COLS], F32, kind="ExternalInput")
    wv = nc.dram_tensor("wv", [D_IN, COLS], F32, kind="ExternalInput")
    bqp = nc.dram_tensor("bqp", [128, NPAIR], F32, kind="ExternalInput")
    bkp = nc.dram_tensor("bkp", [128, NPAIR], F32, kind="ExternalInput")
    bvb = nc.dram_tensor("bvb", [128, COLS], F32, kind="ExternalInput")
    out = nc.dram_tensor("out", [S, COLS], F32, kind="ExternalOutput")

    from contextlib import ExitStack

    with tc_cls(nc) as tc:
        with ExitStack() as ctx:
            ec = ctx.enter_context
            cpool = ec(tc.tile_pool(name="const", bufs=1))
            wpool = ec(tc.tile_pool(name="wsb", bufs=1))
            xstage_pool = ec(tc.tile_pool(name="xstage", bufs=3))
            xtq_pool = ec(tc.tile_pool(name="xtq", bufs=2))
            xtkv_pool = ec(tc.tile_pool(name="xtkv", bufs=2))
            ktv_pool = ec(tc.tile_pool(name="ktv", bufs=1))
            qt_pool = ec(tc.tile_pool(name="qt", bufs=2))
            pt_pool = ec(tc.tile_pool(name="pt", bufs=4))
            outt_pool = ec(tc.tile_pool(name="outt", bufs=2))
            small_pool = ec(tc.tile_pool(name="small", bufs=2))
            ob_pool = ec(tc.tile_pool(name="ob", bufs=2))
            sc_ps = ec(tc.tile_pool(name="psc", bufs=2, space="PSUM"))
            proj_ps = ec(tc.tile_pool(name="pproj", bufs=2, space="PSUM"))
            pv_ps = ec(tc.tile_pool(name="ppv", bufs=2, space="PSUM"))

            # --- constants ---
            bqp_sb = cpool.tile([128, NPAIR], F32, tag="bqp")
            bkp_sb = cpool.tile([128, NPAIR], F32, tag="bkp")
            bvb_sb = cpool.tile([128, COLS], F32, tag="bvb")

            def load_biases():
                # on the Pool ring, emitted mid-stream so these transfers
                # don't delay the k0/q0 critical path on the shared DMA device
                nc.gpsimd.dma_start(bkp_sb[:], bkp[:])
                nc.gpsimd.dma_start(bqp_sb[:], bqp[:])
                nc.gpsimd.dma_start(bvb_sb[:], bvb[:])

            for _rep in range(time_reps):
                # --- weights: SWDGE cast-DMA fp32->bf16 (Pool ring),
                # emitted just-in-time so the Pool FIFO interleaves them
                # with the x block loads ---
                w_sb = {}

                def load_w(name, wd, lo=0, hi=COLS):
                    # column-sliced so the p=0 slice (all the critical path
                    # needs) can transfer before the rest queues up
                    wsb = w_sb.get(name)
                    if wsb is None:
                        wsb = wpool.tile([128, NDC, COLS], BF16, tag=f"w{name}")
                        w_sb[name] = wsb
                    nc.gpsimd.dma_start(
                        wsb[:, :, lo:hi],
                        wd[:, lo:hi].rearrange("(c p) n -> p c n", p=128),
                    )

                def load_xt_block(xdram, dest, blk, pool):
                    # SWDGE cast-DMA the whole 512-token block fp32->bf16 in
                    # one trigger (Pool ring), then xbar transpose per
                    # 128-token chunk to d-major, alternating SP/DVE rings.
                    st = pool.tile([128, 4, D_IN], BF16, tag="xst")
                    nc.gpsimd.dma_start(
                        st[:],
                        xdram[blk * QB : (blk + 1) * QB, :].rearrange(
                            "(c p) n -> p c n", p=128
                        ),
                    )
                    for i in range(4):
                        eng = nc.sync
                        eng.dma_start(
                            dest[:, :, i * 128 : (i + 1) * 128],
                            st[:, i, :],
                            transpose=True,
                        )

                kT_blk, v_blk, qT_all = {}, {}, {}

                def k_load(blk):
                    xtk = xtkv_pool.tile([128, NDC, QB], BF16, tag="xtkv")
                    load_xt_block(xk, xtk, blk, xstage_pool)
                    kT = ktv_pool.tile([128, NPAIR, QB], BF16, tag=f"kT{blk}")
                    kT_blk[blk] = kT
                    return xtk, kT

                def k_proj(xtk, kT, plist):
                    for p in plist:
                        ps = proj_ps.tile([128, 512], F32, tag="proj")
                        for dc in range(NDC):
                            nc.tensor.matmul(
                                ps[:],
                                w_sb["k"][:, dc, p * 128 : (p + 1) * 128],
                                xtk[:, dc, :],
                                start=(dc == 0),
                                stop=(dc == NDC - 1),
                            )
                        nc.vector.tensor_scalar_add(
                            kT[:, p, :], ps[:], bkp_sb[:, p : p + 1]
                        )

                def k_ingest(blk):
                    xtk, kT = k_load(blk)
                    k_proj(xtk, kT, range(NPAIR))

                def v_ingest(blk):
                    xtv = xtkv_pool.tile([128, NDC, QB], BF16, tag="xtkv")
                    load_xt_block(xv, xtv, blk, xstage_pool)
                    vb = ktv_pool.tile([128, 4, 8, HD + 1], BF16, tag=f"v{blk}")
                    v_blk[blk] = vb
                    nc.gpsimd.memset(vb[:, :, :, HD : HD + 1], 1.0)
                    for tci in range(4):
                        ps = proj_ps.tile([128, 512], F32, tag="proj")
                        for dc in range(NDC):
                            nc.tensor.matmul(
                                ps[:],
                                xtv[:, dc, tci * 128 : (tci + 1) * 128],
                                w_sb["v"][:, dc, :],
                                start=(dc == 0),
                                stop=(dc == NDC - 1),
                            )
                        nc.vector.tensor_add(
                            vb[:, tci, :, 0:HD],
                            ps[:].rearrange("p (h d) -> p h d", h=8),
                            bvb_sb[:].rearrange("p (h d) -> p h d", h=8),
                        )

                def q_load(blk):
                    xtqb = xtq_pool.tile([128, NDC, QB], BF16, tag="xtq")
                    load_xt_block(xq, xtqb, blk, xstage_pool)
                    qT = qt_pool.tile([128, NPAIR, QB], BF16, tag="qt", name=f"qT{blk}")
                    qT_all[blk] = qT
                    return xtqb, qT

                def q_proj(xtqb, qT, plist):
                    for p in plist:
                        ps = proj_ps.tile([128, 512], F32, tag="proj")
                        for dc in range(NDC):
                            nc.tensor.matmul(
                                ps[:],
                                w_sb["q"][:, dc, p * 128 : (p + 1) * 128],
                                xtqb[:, dc, :],
                                start=(dc == 0),
                                stop=(dc == NDC - 1),
                            )
                        nc.vector.tensor_scalar_add(
                            qT[:, p, :], ps[:], bqp_sb[:, p : p + 1]
                        )

                def q_ingest(blk):
                    xtqb, qT = q_load(blk)
                    q_proj(xtqb, qT, range(NPAIR))

                NG = NKC // 2  # 8 k-chunk pairs

                def scores_grp(j, p, g, qT_blk, pTa, pTb):
                    kc0 = 2 * g
                    scA = sc_ps.tile([128, 1024], F32, tag="sc")
                    scB = sc_ps.tile([128, 1024], F32, tag="sc")
                    for u in range(2):
                        kc = kc0 + u
                        nc.tensor.matmul(
                            scA[:, u * 512 : (u + 1) * 512],
                            kT_blk[kc // 4][0:64, p, (kc % 4) * 128 : (kc % 4 + 1) * 128],
                            qT_blk[0:64, p, :],
                            start=True,
                            stop=True,
                        )
                        nc.tensor.matmul(
                            scB[:, u * 512 : (u + 1) * 512],
                            kT_blk[kc // 4][64:128, p, (kc % 4) * 128 : (kc % 4 + 1) * 128],
                            qT_blk[64:128, p, :],
                            tile_position=(64, 0),
                            start=True,
                            stop=True,
                        )
                    ew = 1024 // 8 if ablate == "exp_narrow" else 1024
                    nc.scalar.activation(
                        pTa[:, kc0 * 512 : kc0 * 512 + ew],
                        scA[:, 0:ew], EXP, scale=0.125,
                    )
                    nc.scalar.activation(
                        pTb[:, kc0 * 512 : kc0 * 512 + ew],
                        scB[:, 0:ew], EXP, scale=0.125,
                    )

                def pv_grp(p, g, pTa, pTb, psO_a, psO_b):
                    kc0 = 2 * g
                    for u in range(2):
                        kc = kc0 + u
                        if ablate == "pv_lite" and kc not in (0, NKC - 1):
                            continue
                        st = (kc == 0) or ablate == "pv_lite"
                        sp = (kc == NKC - 1) or ablate == "pv_lite"
                        for pT, psO, h in ((pTa, psO_a, 2 * p), (pTb, psO_b, 2 * p + 1)):
                            nc.tensor.matmul(
                                psO[0:65, :],
                                v_blk[kc // 4][:, kc % 4, h, :],
                                pT[:, kc * 512 : (kc + 1) * 512],
                                start=st,
                                stop=sp,
                            )

                def finalize_pair(j, p, psO_a, psO_b):
                    # token-major via the DMA xbar (fp16 pass-through, ~0.05%
                    # rel err) instead of PE transposes; psum copies on Pool
                    # to keep DVE light.
                    with nc.allow_low_precision(reason="fp16 xbar finalize"):
                        outT = outt_pool.tile([128, QB], FP16, tag="outT")
                        nc.gpsimd.tensor_copy(outT[0:64, :], psO_a[0:64, :])
                        nc.gpsimd.tensor_copy(outT[64:128, :], psO_b[0:64, :])
                        dn = outt_pool.tile([16, QB], FP16, tag="dn")
                        nc.gpsimd.memset(dn[:], 1.0)
                        nc.gpsimd.tensor_copy(dn[0:1, :], psO_a[64:65, :])
                        nc.gpsimd.tensor_copy(dn[1:2, :], psO_b[64:65, :])
                        obT = ob_pool.tile([128, 4, 128], FP16, tag="obT")
                        nc.sync.dma_start(obT[:], outT[:], transpose=True)
                        dnT = ob_pool.tile([128, 4, 16], FP16, tag="dnT")
                        nc.sync.dma_start(dnT[:], dn[:], transpose=True)
                        rT = ob_pool.tile([128, 4, 2], F32, tag="rT")
                        nc.vector.reciprocal(rT[:], dnT[:, :, 0:2])
                        ob = ob_pool.tile([128, 4, 128], F32, tag="ob")
                        for tci in range(4):
                            nc.vector.tensor_scalar_mul(
                                ob[:, tci, 0:64], obT[:, tci, 0:64], rT[:, tci, 0:1]
                            )
                            nc.vector.tensor_scalar_mul(
                                ob[:, tci, 64:128], obT[:, tci, 64:128], rT[:, tci, 1:2]
                            )
                        nc.sync.dma_start(
                            out[j * QB : (j + 1) * QB, p * 128 : (p + 1) * 128]
                            .rearrange("(tb t) c -> t tb c", t=128),
                            ob[:],
                        )

                def attention_pair(j, p, qT_blk, hooks=None, pre=None):
                    # software-pipelined: emit scores one group ahead of PV so
                    # the in-order PE stream never blocks on the current
                    # group's exp. `hooks[g]` emits ingest work after pv(g),
                    # `pre` right after the first two score groups — the
                    # in-order PE stream then interleaves prologue
                    # projections with this pair's attention.
                    hooks = hooks or {}
                    pTa = pt_pool.tile([128, NKC * 512], BF16, tag="pt", name=f"pTa_{j}_{p}")
                    pTb = pt_pool.tile([128, NKC * 512], BF16, tag="pt", name=f"pTb_{j}_{p}")
                    psO_a = pv_ps.tile([128, 512], F32, tag="pv", name=f"psOa_{j}_{p}")
                    psO_b = pv_ps.tile([128, 512], F32, tag="pv", name=f"psOb_{j}_{p}")
                    scores_grp(j, p, 0, qT_blk, pTa, pTb)
                    scores_grp(j, p, 1, qT_blk, pTa, pTb)
                    if pre is not None:
                        pre()
                    for g in range(NG):
                        if g in hooks:
                            hooks[g]()
                        if g + 2 < NG:
                            scores_grp(j, p, g + 2, qT_blk, pTa, pTb)
                        pv_grp(p, g, pTa, pTb, psO_a, psO_b)
                    finalize_pair(j, p, psO_a, psO_b)

                # --- schedule: p=0 slices of wk/wq + k0/q0 transfer first so
                # pair (0,0)'s first scores start the ACT exp stream ~18us
                # in; the rest of the weights, v0, and k/v blocks 1-3 ingest
                # inside pair (0,0)'s group sequence (group g needs k/v block
                # g//2); q blocks pipeline one j ahead. ---
                load_w("k", wk, 0, 128)
                xtk0, kT0 = k_load(0)
                load_w("q", wq, 0, 128)
                xtq0, qT0 = q_load(0)
                load_biases()
                k_proj(xtk0, kT0, [0])
                q_proj(xtq0, qT0, [0])

                def rest_of_prologue():
                    load_w("k", wk, 128, COLS)
                    load_w("q", wq, 128, COLS)
                    k_proj(xtk0, kT0, [1, 2, 3])
                    q_proj(xtq0, qT0, [1, 2, 3])
                    load_w("v", wv)
                    v_ingest(0)

                if prologue_only:
                    rest_of_prologue()
                    for blk in range(1, 4):
                        k_ingest(blk)
                        v_ingest(blk)
                    dummy = ob_pool.tile([128, 128], F32, tag="ob2")
                    nc.vector.tensor_copy(dummy[:], kT_blk[3][:, 3, 0:128])
                    nc.gpsimd.dma_start(out[0:128, 0:128], dummy[:])
                else:
                    def kv(blk):
                        return lambda: (k_ingest(blk), v_ingest(blk))

                    for j in range(NQB):
                        for p in range(NPAIR):
                            hooks, pre = None, None
                            if j == 0 and p == 0:
                                pre = rest_of_prologue
                                hooks = {0: kv(1), 2: kv(2), 4: kv(3)}
                            elif p == 3 and j + 1 < NQB:
                                hooks = {0: lambda jn=j + 1: q_ingest(jn)}
                            attention_pair(j, p, qT_all[j], hooks, pre)

    return nc


def _shard_inputs(Q, V, K, wq, bq, wk, bk, wv, bv):
    in_maps = []
    for c in range(N_CORES):
        b, half = c // 2, c % 2
        lo = half * COLS
        bq_s, bk_s, bv_s = bq[lo : lo + COLS], bk[lo : lo + COLS], bv[lo : lo + COLS]
        in_maps.append(
            {
                "xq": np.ascontiguousarray(Q[b]),
                "xk": np.ascontiguousarray(K[b]),
                "xv": np.ascontiguousarray(V[b]),
                "wq": np.ascontiguousarray(wq[:, lo : lo + COLS]),
                "wk": np.ascontiguousarray(wk[:, lo : lo + COLS]),
                "wv": np.ascontiguousarray(wv[:, lo : lo + COLS]),
                "bqp": np.ascontiguousarray(bq_s.reshape(NPAIR, 128).T),
                "bkp": np.ascontiguousarray(bk_s.reshape(NPAIR, 128).T),
                "bvb": np.ascontiguousarray(
                    np.broadcast_to(bv_s, (128, COLS))
                ),
            }
        )
    return in_maps


class SpmdRunner:
    """Compile a Bass program once; run it on 8 cores via PJRT with timing.

    Mirrors bass2jax.run_bass_via_pjrt's multi-core path but keeps the jitted
    executable so repeat executions don't re-trace/re-compile.
    """

    def __init__(self, nc: bass.Bass, n_cores: int = 8):
        import jax
        from jax.sharding import Mesh, PartitionSpec
        from jax.experimental.shard_map import shard_map
        from concourse import bass2jax
        from concourse.bass2jax import _bass_exec_p, install_neuronx_cc_hook

        install_neuronx_cc_hook()
        self.nc = nc
        self.n_cores = n_cores
        self._jax = jax
        self._PartitionSpec = PartitionSpec

        in_names, out_names, out_avals, zero_outs = [], [], [], []
        partition_name = (
            nc.partition_id_tensor.name if nc.partition_id_tensor else None
        )
        for alloc in nc.m.functions[0].allocations:
            if not isinstance(alloc, mybir.MemoryLocationSet):
                continue
            name = alloc.memorylocations[0].name
            if alloc.kind == "ExternalInput":
                if name != partition_name:
                    in_names.append(name)
            elif alloc.kind == "ExternalOutput":
                out_names.append(name)
                shape = tuple(alloc.tensor_shape)
                dtype = mybir.dt.np(alloc.dtype)
                out_avals.append(jax.core.ShapedArray(shape, dtype))
                zero_outs.append(np.zeros(shape, dtype))

        self.in_names = in_names
        self.out_names = out_names
        self.out_avals = out_avals
        self.zero_outs = zero_outs
        n_params = len(in_names)
        n_outs = len(out_avals)
        all_in_names = list(in_names) + list(out_names)
        if partition_name is not None:
            all_in_names.append(partition_name)

        donate = tuple(range(n_params, n_params + n_outs))

        def _body(*args):
            operands = list(args)
            if partition_name is not None:
                operands.append(bass2jax.partition_id_tensor())
            outs = _bass_exec_p.bind(
                *operands,
                out_avals=tuple(out_avals),
                in_names=tuple(all_in_names),
                out_names=tuple(out_names),
                lowering_input_output_aliases=(),
                sim_require_finite=True,
                sim_require_nnan=True,
                nc=nc,
            )
            return tuple(outs)

        devices = jax.devices()[:n_cores]
        self.mesh = Mesh(np.asarray(devices), ("core",))
        in_specs = (PartitionSpec("core"),) * (n_params + n_outs)
        out_specs = (PartitionSpec("core"),) * len(out_names)
        self.sharded = jax.jit(
            shard_map(
                _body,
                mesh=self.mesh,
                in_specs=in_specs,
                out_specs=out_specs,
                check_rep=False,
            ),
            donate_argnums=donate,
            keep_unused=True,
        )

    def run(self, in_maps, iters: int = 1):
        """Returns (results_per_core, best_iter_seconds)."""
        import time as _time

        jax = self._jax
        from jax.sharding import NamedSharding

        sh = NamedSharding(self.mesh, self._PartitionSpec("core"))
        per_core = [
            [np.asarray(m[name]) for name in self.in_names] for m in in_maps
        ]
        concat_in = [
            np.concatenate([per_core[c][i] for c in range(self.n_cores)], axis=0)
            for i in range(len(self.in_names))
        ]
        concat_in = [jax.device_put(a, sh) for a in concat_in]
        for a in concat_in:
            a.block_until_ready()
        times = []
        out_arrs = None
        for _ in range(iters):
            concat_zeros = [
                jax.device_put(
                    np.zeros((self.n_cores * z.shape[0], *z.shape[1:]), z.dtype),
                    sh,
                )
                for z in self.zero_outs
            ]
            for z in concat_zeros:
                z.block_until_ready()
            t0 = _time.perf_counter()
            out_arrs = self.sharded(*concat_in, *concat_zeros)
            for o in out_arrs:
                o.block_until_ready()
            t1 = _time.perf_counter()
            times.append(t1 - t0)
        results = [
            {
                name: np.asarray(out_arrs[i]).reshape(
                    self.n_cores, *self.out_avals[i].shape
                )[c]
                for i, name in enumerate(self.out_names)
            }
            for c in range(self.n_cores)
        ]
        return results, min(times)


_RUNNER = None


def _get_runner():
    global _RUNNER
    if _RUNNER is None:
        _RUNNER = SpmdRunner(build_core_program(), n_cores=N_CORES)
    return _RUNNER


def kernel(**inputs) -> np.ndarray:
    inputs = {k: np.asarray(v) for k, v in inputs.items()}
    in_maps = _shard_inputs(**inputs)
    runner = _get_runner()
    results, _ = runner.run(in_maps, iters=1)
    out = np.empty((B, S, D_MODEL), np.float32)
    for c in range(N_CORES):
        b, half = c // 2, c % 2
        out[b, :, half * COLS : (half + 1) * COLS] = results[c]["out"]
    return out


# revision 40
# speedup vs baseline: 1.5945x; 1.5945x over previous
"""Multi-head attention (B=4, S=2048, D=1024, H=16, HD=64) on 8 TRN2 cores.

Sharding: batch (4) x head-halves (2) -> 8 cores; core c handles batch c//2
and heads [8*(c%2), 8*(c%2)+8) (512 columns of every projection). No
cross-core communication.

Per-core kernel (Tile framework):
  - X (Q/K/V batch slices) ingested via SWDGE cast-DMA (fp32->bf16 on the
    fly, Pool ring), then transposed to d-major with xbar transpose DMAs
    split across the SP and DVE rings.
  - Projections on TensorE in bf16: qT/kT feature-major [head-pair cols,
    toks]; V token-major with a ones column interleaved per head
    ([v_h | 1] is the PV stationary operand, so softmax denominators fall
    out of the PV matmul for free). V is stored fp8e4.
  - scores^T = kT.T @ qT per 128-token k-chunk (bf16), two heads packed in
    the PE array via row tiling.
  - softmax: exp on ScalarE straight out of PSUM (scale=1/8 folded in, no
    max-subtraction -- scores are O(1) by construction), fp8e4 P^T to SBUF.
  - PV: fp8 DoubleRow matmuls contract 256 tokens (a k-chunk pair) per
    instruction at 0.5 cyc/row; out^T[hd+1, q] accumulates in PSUM; row 64
    is the denom.
  - finalize: PE-transpose to token-major, multiply by reciprocal denom, DMA.
  - Schedule: K/V blocks ingest+project first; Q blocks pipeline under the
    attention phase so the ACT exp stream (the bottleneck) starts early and
    stays saturated.
"""

import numpy as np

import concourse.bass as bass
import concourse.tile as tile
from concourse import mybir
from concourse.masks import make_identity

B, S, D_IN, D_MODEL, H = 4, 2048, 1024, 1024, 16
HD = D_MODEL // H  # 64
N_CORES = 8
COLS = 512  # per-core projection columns (8 heads)
NPAIR = 4  # head pairs per core
NKC = S // 128  # 16 k-chunks
NDC = D_IN // 128  # 8 d_in chunks
QB = 512  # q block
NQB = S // QB  # 4

F32 = mybir.dt.float32
BF16 = mybir.dt.bfloat16
FP16 = mybir.dt.float16
FP8 = mybir.dt.float8e4
EXP = mybir.ActivationFunctionType.Exp
DR = mybir.MatmulPerfMode.DoubleRow


def _fixup_multi_waits(nc):
    """Split >cap sync waits per instruction into preceding same-engine NoOps.

    This walrus build rejects more than 1 sync wait command per instruction
    (2 for EventSemaphore); Tile's drain/backedge paths can attach one wait
    per live semaphore to a single Drain.
    """
    for fn in nc.m.functions:
        for block in fn.blocks:
            insts = block.instructions
            i = 0
            while i < len(insts):
                inst = insts[i]
                si = inst.sync_info
                cap = 2 if isinstance(inst, mybir.InstEventSemaphore) else 1
                if si is not None and len(si.on_wait) > cap:
                    waits = list(si.on_wait)
                    keep, extra = waits[:cap], waits[cap:]
                    inst.sync_info = mybir.SyncInfo(
                        on_wait=keep, on_update=list(si.on_update)
                    )
                    nops = [
                        mybir.InstNoOp(
                            name=f"{inst.name}_xwait{j}",
                            engine=inst.engine,
                            bass_nofuse=True,
                            sync_info=mybir.SyncInfo(on_wait=[w], on_update=[]),
                        )
                        for j, w in enumerate(extra)
                    ]
                    insts[i:i] = nops
                    i += len(nops)
                i += 1


class _TC(tile.TileContext):
    def __exit__(self, *args):
        ret = super().__exit__(*args)
        _fixup_multi_waits(self.nc)
        return ret


def build_core_program(fixup_waits: bool = True, time_reps: int = 1, prologue_only: bool = False, ablate: str = '', ingest: str = 'hwdge', wmode: str = 'swdge') -> bass.Bass:
    tc_cls = _TC if fixup_waits else tile.TileContext
    nc = bass.Bass()
    xq = nc.dram_tensor("xq", [S, D_IN], F32, kind="ExternalInput")
    xk = nc.dram_tensor("xk", [S, D_IN], F32, kind="ExternalInput")
    xv = nc.dram_tensor("xv", [S, D_IN], F32, kind="ExternalInput")
    wq = nc.dram_tensor("wq", [D_IN, COLS], F32, kind="ExternalInput")
    wk = nc.dram_tensor("wk", [D_IN, COLS], F32, kind="ExternalInput")
    wv = nc.dram_tensor("wv", [D_IN, COLS], F32, kind="ExternalInput")
    bqp = nc.dram_tensor("bqp", [128, NPAIR], F32, kind="ExternalInput")
    bkp = nc.dram_tensor("bkp", [128, NPAIR], F32, kind="ExternalInput")
    bvb = nc.dram_tensor("bvb", [128, COLS], F32, kind="ExternalInput")
    out = nc.dram_tensor("out", [S, COLS], F32, kind="ExternalOutput")

    from contextlib import ExitStack

    with tc_cls(nc) as tc:
        with ExitStack() as ctx:
            ec = ctx.enter_context
            cpool = ec(tc.tile_pool(name="const", bufs=1))
            wpool = ec(tc.tile_pool(name="wsb", bufs=1))
            xstage_pool = ec(tc.tile_pool(name="xstage", bufs=5))
            xbf_pool = ec(tc.tile_pool(name="xbf", bufs=4))
            xtq_pool = ec(tc.tile_pool(name="xtq", bufs=2))
            xtkv_pool = ec(tc.tile_pool(name="xtkv", bufs=2))
            ktv_pool = ec(tc.tile_pool(name="ktv", bufs=1))
            qt_pool = ec(tc.tile_pool(name="qt", bufs=2))
            pt_pool = ec(tc.tile_pool(name="pt", bufs=4))
            outt_pool = ec(tc.tile_pool(name="outt", bufs=2))
            small_pool = ec(tc.tile_pool(name="small", bufs=2))
            ob_pool = ec(tc.tile_pool(name="ob", bufs=2))
            sc_ps = ec(tc.tile_pool(name="psc", bufs=2, space="PSUM"))
            proj_ps = ec(tc.tile_pool(name="pproj", bufs=2, space="PSUM"))
            pv_ps = ec(tc.tile_pool(name="ppv", bufs=2, space="PSUM"))

            # --- constants ---
            bqp_sb = cpool.tile([128, NPAIR], F32, tag="bqp")
            bkp_sb = cpool.tile([128, NPAIR], F32, tag="bkp")
            bvb_sb = cpool.tile([128, COLS], F32, tag="bvb")

            def load_biases():
                # on the Pool ring, emitted mid-stream so these transfers
                # don't delay the k0/q0 critical path on the shared DMA device
                nc.gpsimd.dma_start(bkp_sb[:], bkp[:])
                nc.gpsimd.dma_start(bqp_sb[:], bqp[:])
                nc.gpsimd.dma_start(bvb_sb[:], bvb[:])

            for _rep in range(time_reps):
                # --- weights: SWDGE cast-DMA fp32->bf16 (Pool ring),
                # emitted just-in-time so the Pool FIFO interleaves them
                # with the x block loads ---
                w_sb = {}

                def load_w(name, wd, lo=0, hi=COLS):
                    # column-sliced so the p=0 slice (all the critical path
                    # needs) can transfer before the rest queues up
                    wsb = w_sb.get(name)
                    if wsb is None:
                        wsb = wpool.tile([128, NDC, COLS], BF16, tag=f"w{name}")
                        w_sb[name] = wsb
                    if wmode == 'swdge':
                        nc.gpsimd.dma_start(
                            wsb[:, :, lo:hi],
                            wd[:, lo:hi].rearrange("(c p) n -> p c n", p=128),
                        )
                    else:
                        # fp32 HWDGE load + DVE cast (no conversion DMA);
                        # one shared staging buffer serializes the loads
                        wst = wpool.tile([128, NDC, COLS], F32, tag="wst")
                        nc.sync.dma_start(
                            wst[:, :, lo:hi],
                            wd[:, lo:hi].rearrange("(c p) n -> p c n", p=128),
                        )
                        nc.vector.tensor_copy(
                            wsb[:, :, lo:hi], wst[:, :, lo:hi]
                        )

                def load_xt_block(xdram, dest, blk, pool):
                    if ingest == 'swdge':
                        # SWDGE cast-DMA the whole 512-token block fp32->bf16
                        # in one trigger (Pool ring), then xbar transpose per
                        # 128-token chunk to d-major on the SP ring.
                        st = pool.tile([128, 4, D_IN], BF16, tag="xst")
                        nc.gpsimd.dma_start(
                            st[:],
                            xdram[blk * QB : (blk + 1) * QB, :].rearrange(
                                "(c p) n -> p c n", p=128
                            ),
                        )
                        for i in range(4):
                            nc.sync.dma_start(
                                dest[:, :, i * 128 : (i + 1) * 128],
                                st[:, i, :],
                                transpose=True,
                            )
                        return
                    # hwdge: fp32 loads on the SP HWDGE ring, casts split
                    # across GpSimd/DVE, then xbar transposes (SP ring).
                    xbs = []
                    for i in range(4):
                        tci = blk * 4 + i
                        st = pool.tile([128, D_IN], F32, tag="xstf")
                        nc.sync.dma_start(
                            st[:], xdram[tci * 128 : (tci + 1) * 128, :]
                        )
                        xb = xbf_pool.tile([128, D_IN], BF16, tag="xbf")
                        eng = nc.gpsimd if i % 2 == 0 else nc.vector
                        eng.tensor_copy(xb[:], st[:])
                        xbs.append(xb)
                    for i, xb in enumerate(xbs):
                        nc.sync.dma_start(
                            dest[:, :, i * 128 : (i + 1) * 128],
                            xb[:],
                            transpose=True,
                        )

                kT_blk, v_blk, qT_all = {}, {}, {}

                def k_load(blk):
                    xtk = xtkv_pool.tile([128, NDC, QB], BF16, tag="xtkv")
                    load_xt_block(xk, xtk, blk, xstage_pool)
                    kT = ktv_pool.tile([128, NPAIR, QB], BF16, tag=f"kT{blk}")
                    kT_blk[blk] = kT
                    return xtk, kT

                def k_proj(xtk, kT, plist):
                    for p in plist:
                        ps = proj_ps.tile([128, 512], F32, tag="proj")
                        for dc in range(NDC):
                            nc.tensor.matmul(
                                ps[:],
                                w_sb["k"][:, dc, p * 128 : (p + 1) * 128],
                                xtk[:, dc, :],
                                start=(dc == 0),
                                stop=(dc == NDC - 1),
                            )
                        nc.vector.tensor_scalar_add(
                            kT[:, p, :], ps[:], bkp_sb[:, p : p + 1]
                        )

                def k_ingest(blk):
                    xtk, kT = k_load(blk)
                    k_proj(xtk, kT, range(NPAIR))

                def v_ingest(blk):
                    xtv = xtkv_pool.tile([128, NDC, QB], BF16, tag="xtkv")
                    load_xt_block(xv, xtv, blk, xstage_pool)
                    vb = ktv_pool.tile([128, 4, 8, HD + 1], BF16, tag=f"v{blk}")
                    v_blk[blk] = vb
                    nc.gpsimd.memset(vb[:, :, :, HD : HD + 1], 1.0)
                    for tci in range(4):
                        ps = proj_ps.tile([128, 512], F32, tag="proj")
                        for dc in range(NDC):
                            nc.tensor.matmul(
                                ps[:],
                                xtv[:, dc, tci * 128 : (tci + 1) * 128],
                                w_sb["v"][:, dc, :],
                                start=(dc == 0),
                                stop=(dc == NDC - 1),
                            )
                        nc.vector.tensor_add(
                            vb[:, tci, :, 0:HD],
                            ps[:].rearrange("p (h d) -> p h d", h=8),
                            bvb_sb[:].rearrange("p (h d) -> p h d", h=8),
                        )

                def q_load(blk):
                    xtqb = xtq_pool.tile([128, NDC, QB], BF16, tag="xtq")
                    load_xt_block(xq, xtqb, blk, xstage_pool)
                    qT = qt_pool.tile([128, NPAIR, QB], BF16, tag="qt", name=f"qT{blk}")
                    qT_all[blk] = qT
                    return xtqb, qT

                def q_proj(xtqb, qT, plist):
                    for p in plist:
                        ps = proj_ps.tile([128, 512], F32, tag="proj")
                        for dc in range(NDC):
                            nc.tensor.matmul(
                                ps[:],
                                w_sb["q"][:, dc, p * 128 : (p + 1) * 128],
                                xtqb[:, dc, :],
                                start=(dc == 0),
                                stop=(dc == NDC - 1),
                            )
                        nc.vector.tensor_scalar_add(
                            qT[:, p, :], ps[:], bqp_sb[:, p : p + 1]
                        )

                def q_ingest(blk):
                    xtqb, qT = q_load(blk)
                    q_proj(xtqb, qT, range(NPAIR))

                NG = NKC // 2  # 8 k-chunk pairs

                def scores_grp(j, p, g, qT_blk, pTa, pTb):
                    kc0 = 2 * g
                    scA = sc_ps.tile([128, 1024], F32, tag="sc")
                    scB = sc_ps.tile([128, 1024], F32, tag="sc")
                    for u in range(2):
                        kc = kc0 + u
                        nc.tensor.matmul(
                            scA[:, u * 512 : (u + 1) * 512],
                            kT_blk[kc // 4][0:64, p, (kc % 4) * 128 : (kc % 4 + 1) * 128],
                            qT_blk[0:64, p, :],
                            start=True,
                            stop=True,
                        )
                        nc.tensor.matmul(
                            scB[:, u * 512 : (u + 1) * 512],
                            kT_blk[kc // 4][64:128, p, (kc % 4) * 128 : (kc % 4 + 1) * 128],
                            qT_blk[64:128, p, :],
                            tile_position=(64, 0),
                            start=True,
                            stop=True,
                        )
                    ew = 1024 // 8 if ablate == "exp_narrow" else 1024
                    nc.scalar.activation(
                        pTa[:, kc0 * 512 : kc0 * 512 + ew],
                        scA[:, 0:ew], EXP, scale=0.125,
                    )
                    nc.scalar.activation(
                        pTb[:, kc0 * 512 : kc0 * 512 + ew],
                        scB[:, 0:ew], EXP, scale=0.125,
                    )

                def pv_grp(p, g, pTa, pTb, psO_a, psO_b):
                    kc0 = 2 * g
                    for u in range(2):
                        kc = kc0 + u
                        if ablate == "pv_lite" and kc not in (0, NKC - 1):
                            continue
                        st = (kc == 0) or ablate == "pv_lite"
                        sp = (kc == NKC - 1) or ablate == "pv_lite"
                        for pT, psO, h in ((pTa, psO_a, 2 * p), (pTb, psO_b, 2 * p + 1)):
                            nc.tensor.matmul(
                                psO[0:65, :],
                                v_blk[kc // 4][:, kc % 4, h, :],
                                pT[:, kc * 512 : (kc + 1) * 512],
                                start=st,
                                stop=sp,
                            )

                def finalize_pair(j, p, psO_a, psO_b):
                    # token-major via the DMA xbar (fp16 pass-through, ~0.05%
                    # rel err) instead of PE transposes; psum copies on Pool
                    # to keep DVE light.
                    with nc.allow_low_precision(reason="fp16 xbar finalize"):
                        outT = outt_pool.tile([128, QB], FP16, tag="outT")
                        nc.vector.tensor_copy(outT[0:64, :], psO_a[0:64, :])
                        nc.vector.tensor_copy(outT[64:128, :], psO_b[0:64, :])
                        dn = outt_pool.tile([48, QB], FP16, tag="dn")
                        nc.gpsimd.memset(dn[:], 1.0)
                        nc.vector.tensor_copy(dn[0:1, :], psO_a[64:65, :])
                        nc.vector.tensor_copy(dn[32:33, :], psO_b[64:65, :])
                        obT = ob_pool.tile([128, 4, 128], FP16, tag="obT")
                        nc.sync.dma_start(obT[:], outT[:], transpose=True)
                        dnT = ob_pool.tile([128, 4, 48], FP16, tag="dnT")
                        nc.sync.dma_start(dnT[:], dn[:], transpose=True)
                        rT = ob_pool.tile([128, 4, 2], F32, tag="rT")
                        nc.vector.reciprocal(rT[:, :, 0:1], dnT[:, :, 0:1])
                        nc.vector.reciprocal(rT[:, :, 1:2], dnT[:, :, 32:33])
                        ob = ob_pool.tile([128, 4, 128], F32, tag="ob")
                        for tci in range(4):
                            nc.vector.tensor_scalar_mul(
                                ob[:, tci, 0:64], obT[:, tci, 0:64], rT[:, tci, 0:1]
                            )
                            nc.vector.tensor_scalar_mul(
                                ob[:, tci, 64:128], obT[:, tci, 64:128], rT[:, tci, 1:2]
                            )
                        nc.gpsimd.dma_start(
                            out[j * QB : (j + 1) * QB, p * 128 : (p + 1) * 128]
                            .rearrange("(tb t) c -> t tb c", t=128),
                            ob[:],
                        )

                def attention_pair(j, p, qT_blk, hooks=None, pre=None):
                    # software-pipelined: emit scores one group ahead of PV so
                    # the in-order PE stream never blocks on the current
                    # group's exp. `hooks[g]` emits ingest work after pv(g),
                    # `pre` right after the first two score groups — the
                    # in-order PE stream then interleaves prologue
                    # projections with this pair's attention.
                    hooks = hooks or {}
                    pTa = pt_pool.tile([128, NKC * 512], BF16, tag="pt", name=f"pTa_{j}_{p}")
                    pTb = pt_pool.tile([128, NKC * 512], BF16, tag="pt", name=f"pTb_{j}_{p}")
                    psO_a = pv_ps.tile([128, 512], F32, tag="pv", name=f"psOa_{j}_{p}")
                    psO_b = pv_ps.tile([128, 512], F32, tag="pv", name=f"psOb_{j}_{p}")
                    scores_grp(j, p, 0, qT_blk, pTa, pTb)
                    scores_grp(j, p, 1, qT_blk, pTa, pTb)
                    if pre is not None:
                        pre()
                    for g in range(NG):
                        if g in hooks:
                            hooks[g]()
                        if g + 2 < NG:
                            scores_grp(j, p, g + 2, qT_blk, pTa, pTb)
                        pv_grp(p, g, pTa, pTb, psO_a, psO_b)
                    finalize_pair(j, p, psO_a, psO_b)

                # --- schedule: p=0 slices of wk/wq + k0/q0 transfer first so
                # pair (0,0)'s first scores start the ACT exp stream ~18us
                # in; the rest of the weights, v0, and k/v blocks 1-3 ingest
                # inside pair (0,0)'s group sequence (group g needs k/v block
                # g//2); q blocks pipeline one j ahead. ---
                load_w("k", wk, 0, 128)
                xtk0, kT0 = k_load(0)
                load_w("q", wq, 0, 128)
                xtq0, qT0 = q_load(0)
                load_biases()
                k_proj(xtk0, kT0, [0])
                q_proj(xtq0, qT0, [0])

                def rest_of_prologue():
                    load_w("k", wk, 128, COLS)
                    load_w("q", wq, 128, COLS)
                    k_proj(xtk0, kT0, [1, 2, 3])
                    q_proj(xtq0, qT0, [1, 2, 3])
                    load_w("v", wv)
                    v_ingest(0)

                if prologue_only:
                    rest_of_prologue()
                    for blk in range(1, 4):
                        k_ingest(blk)
                        v_ingest(blk)
                    dummy = ob_pool.tile([128, 128], F32, tag="ob2")
                    nc.vector.tensor_copy(dummy[:], kT_blk[3][:, 3, 0:128])
                    nc.gpsimd.dma_start(out[0:128, 0:128], dummy[:])
                else:
                    def kv(blk):
                        return lambda: (k_ingest(blk), v_ingest(blk))

                    for j in range(NQB):
                        for p in range(NPAIR):
                            hooks, pre = None, None
                            if j == 0 and p == 0:
                                pre = rest_of_prologue
                                hooks = {0: kv(1), 2: kv(2), 4: kv(3)}
                            elif p == 3 and j + 1 < NQB:
                                hooks = {0: lambda jn=j + 1: q_ingest(jn)}
                            attention_pair(j, p, qT_all[j], hooks, pre)

    return nc


def _shard_inputs(Q, V, K, wq, bq, wk, bk, wv, bv):
    in_maps = []
    for c in range(N_CORES):
        b, half = c // 2, c % 2
        lo = half * COLS
        bq_s, bk_s, bv_s = bq[lo : lo + COLS], bk[lo : lo + COLS], bv[lo : lo + COLS]
        in_maps.append(
            {
                "xq": np.ascontiguousarray(Q[b]),
                "xk": np.ascontiguousarray(K[b]),
                "xv": np.ascontiguousarray(V[b]),
                "wq": np.ascontiguousarray(wq[:, lo : lo + COLS]),
                "wk": np.ascontiguousarray(wk[:, lo : lo + COLS]),
                "wv": np.ascontiguousarray(wv[:, lo : lo + COLS]),
                "bqp": np.ascontiguousarray(bq_s.reshape(NPAIR, 128).T),
                "bkp": np.ascontiguousarray(bk_s.reshape(NPAIR, 128).T),
                "bvb": np.ascontiguousarray(
                    np.broadcast_to(bv_s, (128, COLS))
                ),
            }
        )
    return in_maps


class SpmdRunner:
    """Compile a Bass program once; run it on 8 cores via PJRT with timing.

    Mirrors bass2jax.run_bass_via_pjrt's multi-core path but keeps the jitted
    executable so repeat executions don't re-trace/re-compile.
    """

    def __init__(self, nc: bass.Bass, n_cores: int = 8):
        import jax
        from jax.sharding import Mesh, PartitionSpec
        from jax.experimental.shard_map import shard_map
        from concourse import bass2jax
        from concourse.bass2jax import _bass_exec_p, install_neuronx_cc_hook

        install_neuronx_cc_hook()
        self.nc = nc
        self.n_cores = n_cores
        self._jax = jax
        self._PartitionSpec = PartitionSpec

        in_names, out_names, out_avals, zero_outs = [], [], [], []
        partition_name = (
            nc.partition_id_tensor.name if nc.partition_id_tensor else None
        )
        for alloc in nc.m.functions[0].allocations:
            if not isinstance(alloc, mybir.MemoryLocationSet):
                continue
            name = alloc.memorylocations[0].name
            if alloc.kind == "ExternalInput":
                if name != partition_name:
                    in_names.append(name)
            elif alloc.kind == "ExternalOutput":
                out_names.append(name)
                shape = tuple(alloc.tensor_shape)
                dtype = mybir.dt.np(alloc.dtype)
                out_avals.append(jax.core.ShapedArray(shape, dtype))
                zero_outs.append(np.zeros(shape, dtype))

        self.in_names = in_names
        self.out_names = out_names
        self.out_avals = out_avals
        self.zero_outs = zero_outs
        n_params = len(in_names)
        n_outs = len(out_avals)
        all_in_names = list(in_names) + list(out_names)
        if partition_name is not None:
            all_in_names.append(partition_name)

        donate = tuple(range(n_params, n_params + n_outs))

        def _body(*args):
            operands = list(args)
            if partition_name is not None:
                operands.append(bass2jax.partition_id_tensor())
            outs = _bass_exec_p.bind(
                *operands,
                out_avals=tuple(out_avals),
                in_names=tuple(all_in_names),
                out_names=tuple(out_names),
                lowering_input_output_aliases=(),
                sim_require_finite=True,
                sim_require_nnan=True,
                nc=nc,
            )
            return tuple(outs)

        devices = jax.devices()[:n_cores]
        self.mesh = Mesh(np.asarray(devices), ("core",))
        in_specs = (PartitionSpec("core"),) * (n_params + n_outs)
        out_specs = (PartitionSpec("core"),) * len(out_names)
        self.sharded = jax.jit(
            shard_map(
                _body,
                mesh=self.mesh,
                in_specs=in_specs,
                out_specs=out_specs,
                check_rep=False,
            ),
            donate_argnums=donate,
            keep_unused=True,
        )

    def run(self, in_maps, iters: int = 1):
        """Returns (results_per_core, best_iter_seconds)."""
        import time as _time

        jax = self._jax
        from jax.sharding import NamedSharding

        sh = NamedSharding(self.mesh, self._PartitionSpec("core"))
        per_core = [
            [np.asarray(m[name]) for name in self.in_names] for m in in_maps
        ]
        concat_in = [
            np.concatenate([per_core[c][i] for c in range(self.n_cores)], axis=0)
            for i in range(len(self.in_names))
        ]
        concat_in = [jax.device_put(a, sh) for a in concat_in]
        for a in concat_in:
            a.block_until_ready()
        times = []
        out_arrs = None
        for _ in range(iters):
            concat_zeros = [
                jax.device_put(
                    np.zeros((self.n_cores * z.shape[0], *z.shape[1:]), z.dtype),
                    sh,
                )
                for z in self.zero_outs
            ]
            for z in concat_zeros:
                z.block_until_ready()
            t0 = _time.perf_counter()
            out_arrs = self.sharded(*concat_in, *concat_zeros)
            for o in out_arrs:
                o.block_until_ready()
            t1 = _time.perf_counter()
            times.append(t1 - t0)
        results = [
            {
                name: np.asarray(out_arrs[i]).reshape(
                    self.n_cores, *self.out_avals[i].shape
                )[c]
                for i, name in enumerate(self.out_names)
            }
            for c in range(self.n_cores)
        ]
        return results, min(times)


_RUNNER = None


def _get_runner():
    global _RUNNER
    if _RUNNER is None:
        _RUNNER = SpmdRunner(build_core_program(), n_cores=N_CORES)
    return _RUNNER


def kernel(**inputs) -> np.ndarray:
    inputs = {k: np.asarray(v) for k, v in inputs.items()}
    in_maps = _shard_inputs(**inputs)
    runner = _get_runner()
    results, _ = runner.run(in_maps, iters=1)
    out = np.empty((B, S, D_MODEL), np.float32)
    for c in range(N_CORES):
        b, half = c // 2, c % 2
        out[b, :, half * COLS : (half + 1) * COLS] = results[c]["out"]
    return out
